# revision 8
# baseline (speedup 1.0000x reference)
"""GCGRU cell (order-2 graph diffusion GRU) Trainium2 Bass kernel.

Strategy: data-parallel over batch (B=16 -> 2 batches per core x 8 cores).
The 4000x4000 adjacency is kept RESIDENT in SBUF as fp8 (adjT scaled by 2^11
into e4m3 range, chunk-major [128, 32 chunks, 4000]), so HBM sees it exactly
once per core (~16MB) instead of once per diffusion stage (4x33MB fp16).

All four graph diffusions run as fp8 DoubleRow matmuls (K=256 per chunk pair)
with the activations as the PE-stationary operand and adjacency slabs moving
512 output columns at a time: z1=(A z), z2=(A z1) for the gates, zc1=(A rh),
zc2=(A zc1) for the candidate. Diffused features carry the 2^11 adjacency
scale in fp16/fp8; conv weights are pre-scaled by 2^-11 host-side, and the
second-order stages descale their PSUM copy by 2^-11 on ScalarE.

Chained diffusions need node-major stationaries, so each stage's PSUM bands
are PE-transposed (128-chunk pieces) and cast to fp8 on DVE. Gate and
candidate 1x1 convs consume channel-major copies (fp16 for the dominant
direct terms, fp8 for the small diffused terms), fused per 512-node band:
sigmoid/tanh on ScalarE, r*h / u*h+(1-u)*c combines on DVE, outputs DMA'd
per band. All input casts/layout transforms are done on host in kernel().
"""

import numpy as np
import ml_dtypes

import concourse.bass as bass
from concourse import bacc
import concourse.mybir as mybir
import concourse.tile as tile
from concourse.bass_utils import run_bass_kernel_spmd

# problem constants
B, D_IN, D_H, NN = 16, 32, 64, 4000
NCORES = 8
B_LOC = B // NCORES          # batches per core
C = D_IN + D_H               # 96 channels into each gate conv
BC = B_LOC * C               # 192 batch-channel columns (b-major)
BH = B_LOC * D_H             # 128 stacked batch-hidden rows
NP = 4096                    # node contraction dim padded to 32 chunks
NCH = NP // 128              # 32 node chunks
NJP = NCH // 2               # 16 DoubleRow chunk pairs
SC = 2048.0                  # adjacency pre-scale (2^11) into fp8 range
BAND = 512                   # node band width for PSUM groups / convs
NBAND = (NN + BAND - 1) // BAND   # 8 bands, last is 416 wide

F8 = mybir.dt.float8e4
F16 = mybir.dt.float16
F32 = mybir.dt.float32
DR = mybir.MatmulPerfMode.DoubleRow
NP8 = ml_dtypes.float8_e4m3


def band_w(bi):
    return min(BAND, NN - bi * BAND)


def build_program():
    nc = bacc.Bacc("TRN2", target_bir_lowering=False, debug=False)

    # ---- DRAM I/O (all host-prepped layouts) ----
    # at[g][p, j, m] = adjT[(4g+j)*128+p, m] * SC, fp8, m < 4000
    at_d = nc.dram_tensor("at", [8, 128, 4, NN], F8, kind="ExternalInput").ap()
    # zt[p, j, c] = concat(x,h)[b, ch, j*128+p] fp8, c = b*96+ch
    zt_d = nc.dram_tensor("zt", [128, NCH, BC], F8, kind="ExternalInput").ap()
    xal_d = nc.dram_tensor("xal", [128, NP], F16, kind="ExternalInput").ap()
    hal_d = nc.dram_tensor("hal", [128, NP], F16, kind="ExternalInput").ap()
    wfu_d = nc.dram_tensor("wfu", [3, C, 2 * D_H], F16, kind="ExternalInput").ap()
    wg0x_d = nc.dram_tensor("wg0x", [128, 2 * D_H], F16, kind="ExternalInput").ap()
    wg0h_d = nc.dram_tensor("wg0h", [128, 2 * D_H], F16, kind="ExternalInput").ap()
    wcx0_d = nc.dram_tensor("wcx0", [128, D_H], F16, kind="ExternalInput").ap()
    wcxz_d = nc.dram_tensor("wcxz", [2 * D_H, D_H], F16,
                            kind="ExternalInput").ap()
    wch_d = nc.dram_tensor("wch", [2 * D_H, D_H], F16, kind="ExternalInput").ap()
    wch2_d = nc.dram_tensor("wch2", [D_H, D_H], F16, kind="ExternalInput").ap()
    bfu_d = nc.dram_tensor("bfu", [2 * D_H, 1], F32, kind="ExternalInput").ap()
    bcb_d = nc.dram_tensor("bcb", [2 * D_H, 1], F32, kind="ExternalInput").ap()
    id_d = nc.dram_tensor("idm", [128, 128], F16, kind="ExternalInput").ap()
    out_d = nc.dram_tensor("out", [B_LOC, D_H, NN], F32,
                           kind="ExternalOutput").ap()

    with tile.TileContext(nc) as tc:
        _body(tc, at_d, zt_d, xal_d, hal_d, wfu_d, wg0x_d, wg0h_d,
              wcx0_d, wcxz_d, wch_d, wch2_d, bfu_d, bcb_d, id_d, out_d)
    nc.compile()
    return nc


def _body(tc, at_d, zt_d, xal_d, hal_d, wfu_d, wg0x_d, wg0h_d,
          wcx0_d, wcxz_d, wch_d, wch2_d, bfu_d, bcb_d, id_d, out_d):
    nc = tc.nc
    SIG = mybir.ActivationFunctionType.Sigmoid
    TANH = mybir.ActivationFunctionType.Tanh
    COPY = mybir.ActivationFunctionType.Copy

    with (
        tc.tile_pool(name="const", bufs=1) as cpool,     # weights/bias/idm/at
        tc.tile_pool(name="ztp", bufs=2) as ztpool,      # zt8 <-> z1T8 reuse
        tc.tile_pool(name="act8", bufs=1) as a8pool,     # persistent fp8 acts
        tc.tile_pool(name="perst", bufs=1) as ppool,     # persistent fp16 acts
        tc.tile_pool(name="bnd", bufs=6) as bpool,       # band staging tiles
        tc.tile_pool(name="stg", bufs=3) as stpool,      # combine staging
        tc.tile_pool(name="psmm", bufs=5, space="PSUM") as mmpool,
        tc.tile_pool(name="pscv", bufs=2, space="PSUM") as cvpool,
        tc.tile_pool(name="psx", bufs=1, space="PSUM") as txpool,
    ):
        # ---- persistent loads ----
        idm = cpool.tile([128, 128], F16, tag="idm")
        nc.sync.dma_start(out=idm[:], in_=id_d[:])
        wfu = [cpool.tile([C, 2 * D_H], F16, tag=f"wfu{k}", name=f"wfu{k}")
               for k in range(3)]
        for k in range(3):
            nc.scalar.dma_start(out=wfu[k][:], in_=wfu_d[k])
        wg0x = cpool.tile([128, 2 * D_H], F16, tag="wg0x")
        nc.sync.dma_start(out=wg0x[:], in_=wg0x_d[:])
        wg0h = cpool.tile([128, 2 * D_H], F16, tag="wg0h")
        nc.sync.dma_start(out=wg0h[:], in_=wg0h_d[:])
        wcx0 = cpool.tile([128, D_H], F16, tag="wcx0")
        nc.sync.dma_start(out=wcx0[:], in_=wcx0_d[:])
        wcxz = cpool.tile([2 * D_H, D_H], F16, tag="wcxz")
        nc.sync.dma_start(out=wcxz[:], in_=wcxz_d[:])
        wch = cpool.tile([2 * D_H, D_H], F16, tag="wch")
        nc.sync.dma_start(out=wch[:], in_=wch_d[:])
        wch2 = cpool.tile([D_H, D_H], F16, tag="wch2")
        nc.sync.dma_start(out=wch2[:], in_=wch2_d[:])
        bfu = cpool.tile([2 * D_H, 1], F32, tag="bfu")
        nc.sync.dma_start(out=bfu[:], in_=bfu_d[:])
        bcb = cpool.tile([2 * D_H, 1], F32, tag="bcb")
        nc.sync.dma_start(out=bcb[:], in_=bcb_d[:])
        xall = ppool.tile([128, NP], F16, tag="xall")
        nc.scalar.dma_start(out=xall[:], in_=xal_d[:])
        hall = ppool.tile([128, NP], F16, tag="hall")
        nc.scalar.dma_start(out=hall[:], in_=hal_d[:])

        zt8 = ztpool.tile([128, NCH * BC], F8, tag="ztn", name="zt8")
        nc.sync.dma_start(
            out=zt8[:, :].rearrange("p (j c) -> p j c", j=NCH), in_=zt_d[:])

        # resident adjacency, 8 slab DMAs alternating HWDGE rings
        at8 = cpool.tile([128, NCH * NN], F8, tag="at8")
        at3 = at8[:, :].rearrange("p (j m) -> p j m", j=NCH)
        for g in range(8):
            eng = nc.sync if g % 2 == 0 else nc.scalar
            eng.dma_start(out=at3[:, 4 * g:4 * g + 4, :], in_=at_d[g])

        # persistent activations
        u_st = ppool.tile([BH, NP], F16, tag="u_st")     # u, rows b*64
        z1cm8 = [a8pool.tile([C, NP], F8, tag=f"z1cm{b}", name=f"z1cm{b}")
                 for b in range(B_LOC)]
        # candidate diffused-x features, rows [b0z1x|b0z2x|b1z1x|b1z2x]
        candx = a8pool.tile([128, NP], F8, tag="candx")
        rhz = [a8pool.tile([2 * D_H, NP], F8, tag=f"rhz{b}", name=f"rhz{b}")
               for b in range(B_LOC)]                    # [rh|zc1] rows
        rhT8 = a8pool.tile([128, NCH * BH], F8, tag="rhT8")
        zc1T8 = a8pool.tile([128, NCH * BH], F8, tag="zc1T8")
        z1T8 = ztpool.tile([128, NCH * BC], F8, tag="ztn", name="z1T8")

        zt3 = zt8[:, :].rearrange("p (j c) -> p j c", j=NCH)
        z1T3 = z1T8[:, :].rearrange("p (j c) -> p j c", j=NCH)
        rhT3 = rhT8[:, :].rearrange("p (j c) -> p j c", j=NCH)
        zc1T3 = zc1T8[:, :].rearrange("p (j c) -> p j c", j=NCH)

        # zero the padded node rows (4000:4096 live in chunk 31) of the
        # on-chip-built stationaries so later contractions see zeros; the
        # band transposes overwrite rows 0:32 with real data afterwards
        nc.vector.memset(z1T3[:, 31, :], 0.0)
        nc.vector.memset(rhT3[:, 31, :], 0.0)
        nc.vector.memset(zc1T3[:, 31, :], 0.0)

        def diffuse_bc(src3, bands, psname):
            """DoubleRow diffusion, [*,*,BC]-layout stationary; returns
            per-(band, batch) psum tiles [C, w] accumulated over all nodes."""
            pss = {}
            for bi in bands:
                for b in range(B_LOC):
                    pss[bi, b] = mmpool.tile([C, BAND], F32, tag="mm",
                                             name=f"{psname}_{bi}_{b}")
            for jp in range(NJP):
                for b in range(B_LOC):
                    lhsT = src3[:, 2 * jp:2 * jp + 2, b * C:(b + 1) * C]
                    for bi in bands:
                        w = band_w(bi)
                        nc.tensor.matmul(
                            pss[bi, b][:, 0:w], lhsT=lhsT,
                            rhs=at3[:, 2 * jp:2 * jp + 2,
                                    bi * BAND:bi * BAND + w],
                            start=(jp == 0), stop=(jp == NJP - 1),
                            perf_mode=DR)
            return pss

        def diffuse_bh(src3, bands, psname):
            """DoubleRow diffusion, [*,*,BH]-layout stationary (both batches
            in one tile); returns per-band psum tiles [BH, w]."""
            pss = {}
            for bi in bands:
                pss[bi] = mmpool.tile([BH, BAND], F32, tag="mm",
                                      name=f"{psname}_{bi}")
            for jp in range(NJP):
                lhsT = src3[:, 2 * jp:2 * jp + 2, :]
                for bi in bands:
                    w = band_w(bi)
                    nc.tensor.matmul(
                        pss[bi][:, 0:w], lhsT=lhsT,
                        rhs=at3[:, 2 * jp:2 * jp + 2, bi * BAND:bi * BAND + w],
                        start=(jp == 0), stop=(jp == NJP - 1), perf_mode=DR)
            return pss

        def transpose_band(srcT, r0, rows, bi, dst3, dcol0, dcols):
            """Transpose band staging srcT[r0:r0+rows, 0:w] (f16) into fp8
            node-major dst3[:, j, dcol0:dcol0+dcols] chunk pieces. Full
            128-wide pieces go through the DMA xbar into an f16 staging tile
            (one batched fp8 cast per band); the ragged tail piece uses a PE
            transpose."""
            w = band_w(bi)
            m0 = bi * BAND
            rsl = slice(r0, r0 + rows)
            js = list(range(m0 // 128, (m0 + w + 127) // 128))
            full = [j for j in js if w - (j * 128 - m0) >= 128]
            xbt = bpool.tile([128, 4 * rows], F16, tag="xbt", name="xbt",
                             bufs=4)
            for k, j in enumerate(full):
                off = j * 128 - m0
                eng = nc.sync if j % 2 == 0 else nc.scalar
                eng.dma_start_transpose(xbt[:, k * rows:(k + 1) * rows],
                                        srcT[rsl, off:off + 128])
            nf = len(full)
            nc.vector.tensor_copy(
                out=dst3[:, full[0]:full[0] + nf, dcol0:dcol0 + dcols],
                in_=xbt[:, 0:nf * rows].rearrange("p (k c) -> p k c", k=nf))
            for j in js[nf:]:
                off = j * 128 - m0
                wj = w - off
                pt = txpool.tile([128, rows], F16, tag="tx", name=f"tp{j}")
                nc.tensor.transpose(pt[0:wj, :], srcT[rsl, off:off + wj],
                                    idm[rsl, rsl])
                nc.vector.tensor_copy(
                    out=dst3[0:wj, j, dcol0:dcol0 + dcols], in_=pt[0:wj, :])

        # ================= stage 1: z1 = (A*SC) z =================
        for q in range(NBAND // 2):
            bands = (2 * q, 2 * q + 1)
            pss = diffuse_bc(zt3, bands, "ps1")
            for bi in bands:
                w = band_w(bi)
                msl = slice(bi * BAND, bi * BAND + w)
                for b in range(B_LOC):
                    ps = pss[bi, b]
                    z1b = bpool.tile([C, BAND], F16, tag="bnd", name="z1b")
                    nc.scalar.activation(z1b[:, 0:w], ps[:, 0:w], COPY)
                    nc.vector.tensor_copy(out=z1cm8[b][:, msl], in_=ps[:, 0:w])
                    nc.vector.tensor_copy(
                        out=candx[b * D_H:b * D_H + D_IN, msl],
                        in_=ps[0:D_IN, 0:w])
                    transpose_band(z1b, 0, C, bi, z1T3, b * C, C)

        # ============ stage 2: z2 = (A*SC) z1s, gates, rh ============
        for q in range(NBAND // 2):
            bands = (2 * q, 2 * q + 1)
            pss = diffuse_bc(z1T3, bands, "ps2")
            for bi in bands:
                w = band_w(bi)
                msl = slice(bi * BAND, bi * BAND + w)
                rst = bpool.tile([BH, BAND], F16, tag="bnd", name="rst")
                for b in range(B_LOC):
                    ps = pss[bi, b]
                    # z2s = 2^11 z2 (psum holds 2^22 z2)
                    z2b = bpool.tile([C, BAND], F16, tag="bnd", name="z2b")
                    nc.scalar.activation(z2b[:, 0:w], ps[:, 0:w], COPY,
                                         scale=1.0 / SC)
                    nc.vector.tensor_scalar_mul(
                        candx[b * D_H + D_IN:b * D_H + 2 * D_IN, msl],
                        ps[0:D_IN, 0:w], 1.0 / SC)
                    # gate conv: r|u preact = Wfu0 z + Wfu1' z1s + Wfu2' z2s
                    psg = cvpool.tile([2 * D_H, BAND], F32, tag="cv",
                                      name="psg")
                    gt = ((wg0x[b * D_H:b * D_H + D_IN, :],
                           xall[b * D_H:b * D_H + D_IN, msl]),
                          (wg0h[b * D_H:(b + 1) * D_H, :],
                           hall[b * D_H:(b + 1) * D_H, msl]),
                          (wfu[1], z1cm8[b][:, msl]),
                          (wfu[2], z2b[:, 0:w]))
                    for k, (wt, rhs) in enumerate(gt):
                        nc.tensor.matmul(psg[:, 0:w], lhsT=wt, rhs=rhs,
                                         start=(k == 0),
                                         stop=(k == len(gt) - 1))
                    rows = slice(b * D_H, (b + 1) * D_H)
                    nc.scalar.activation(rst[rows, 0:w], psg[0:D_H, 0:w],
                                         SIG, bias=bfu[0:D_H, :])
                    nc.scalar.activation(u_st[rows, msl],
                                         psg[D_H:2 * D_H, 0:w], SIG,
                                         bias=bfu[D_H:2 * D_H, :])
                if True:  # both batches' r now staged batch-stacked in rst
                    rhb = bpool.tile([BH, BAND], F16, tag="bnd", name="rhb")
                    nc.vector.tensor_mul(out=rhb[:, 0:w], in0=rst[:, 0:w],
                                         in1=hall[:, msl])
                    for b in range(B_LOC):
                        rows = slice(b * D_H, (b + 1) * D_H)
                        nc.vector.tensor_copy(out=rhz[b][0:D_H, msl],
                                              in_=rhb[rows, 0:w])
                        transpose_band(rhb, b * D_H, D_H, bi, rhT3,
                                       b * D_H, D_H)

        # ================= stage 3: zc1 = (A*SC) rh =================
        for hh in range(NBAND // 4):
            bands = tuple(range(4 * hh, 4 * hh + 4))
            pss = diffuse_bh(rhT3, bands, "ps3")
            for bi in bands:
                w = band_w(bi)
                msl = slice(bi * BAND, bi * BAND + w)
                for b in range(B_LOC):
                    rows = slice(b * D_H, (b + 1) * D_H)
                    zc1b = bpool.tile([D_H, BAND], F16, tag="bnd",
                                      name="zc1b")
                    nc.scalar.activation(zc1b[:, 0:w], pss[bi][rows, 0:w],
                                         COPY)
                    nc.vector.tensor_copy(out=rhz[b][D_H:2 * D_H, msl],
                                          in_=pss[bi][rows, 0:w])
                    transpose_band(zc1b, 0, D_H, bi, zc1T3, b * D_H, D_H)

        # ================ stage 4: zc2, candidate, output ================
        for bands in ((0, 1, 2, 3), (4, 5), (6,), (7,)):
            pss = diffuse_bh(zc1T3, bands, "ps4")
            for bi in bands:
                w = band_w(bi)
                msl = slice(bi * BAND, bi * BAND + w)
                psc = cvpool.tile([BH, BAND], F32, tag="cv", name="psc")
                for b in range(B_LOC):
                    rows = slice(b * D_H, (b + 1) * D_H)
                    zc2b = bpool.tile([D_H, BAND], F16, tag="bnd",
                                      name="zc2b")
                    nc.scalar.activation(zc2b[:, 0:w], pss[bi][rows, 0:w],
                                         COPY, scale=1.0 / SC)
                    terms = ((wcx0[b * D_H:b * D_H + D_IN, :],
                              xall[b * D_H:b * D_H + D_IN, msl]),
                             (wcxz[b * D_H:(b + 1) * D_H, :],
                              candx[b * D_H:(b + 1) * D_H, msl]),
                             (wch, rhz[b][:, msl]),
                             (wch2, zc2b[:, 0:w]))
                    for k, (wt, rhs) in enumerate(terms):
                        nc.tensor.matmul(psc[rows, 0:w], lhsT=wt, rhs=rhs,
                                         start=(k == 0),
                                         stop=(k == len(terms) - 1))
                cst = bpool.tile([BH, BAND], F16, tag="bnd", name="cst")
                nc.scalar.activation(cst[:, 0:w], psc[:, 0:w], TANH,
                                     bias=bcb[:, :])
                t1 = stpool.tile([BH, BAND], F32, tag="cmb", name="t1")
                nc.vector.tensor_sub(out=t1[:, 0:w], in0=hall[:, msl],
                                     in1=cst[:, 0:w])
                nc.vector.tensor_mul(out=t1[:, 0:w], in0=u_st[:, msl],
                                     in1=t1[:, 0:w])
                ost = stpool.tile([BH, BAND], F32, tag="cmb", name="ost")
                nc.vector.tensor_add(out=ost[:, 0:w], in0=cst[:, 0:w],
                                     in1=t1[:, 0:w])
                for b in range(B_LOC):
                    nc.scalar.dma_start(
                        out=out_d[b][:, msl],
                        in_=ost[b * D_H:(b + 1) * D_H, 0:w])


# ---- host-side driver ----
_CACHED_NC = None
TRACE = False
TRACE_DIR = None
LAST_RESULTS = None


def _f8(a):
    return np.clip(a, -240.0, 240.0).astype(NP8)


def _host_prep(x, h, adj, Wf, bf, Wu, bu, Wc, bc):
    # adjacency: transpose, scale, pad contraction rows to 4096, fp8,
    # chunk-major groups of 4
    at = np.zeros((NP, NN), dtype=np.float32)
    at[:NN, :] = adj.T * SC
    at8 = _f8(at).reshape(8, 4, 128, NN).transpose(0, 2, 1, 3)
    at8 = np.ascontiguousarray(at8)                       # [8, 128, 4, 4000]
    idm = np.eye(128, dtype=np.float16)

    def wsplit(W, s):  # [D_H, 3C] -> [3, C, D_H] with per-order scales
        WT = np.ascontiguousarray(W.T.reshape(3, C, D_H)).astype(np.float32)
        return WT * np.asarray(s, np.float32)[:, None, None]

    wf3 = wsplit(Wf, [1, 1 / SC, 1 / SC])
    wu3 = wsplit(Wu, [1, 1 / SC, 1 / SC])
    wc3 = wsplit(Wc, [1, 1 / SC, 1 / SC])
    # gate weights packed [96, f64|u64] per order
    wfu = np.concatenate([wf3, wu3], axis=2).astype(np.float16)  # [3, 96, 128]
    # direct-term gate weights split x/h, duplicated per batch row-block so
    # matmul stationary/moving base partitions match (b0 rows 0:*, b1 at 64:*)
    wg0x = np.zeros((128, 2 * D_H), dtype=np.float16)
    wg0x[0:D_IN] = wg0x[D_H:D_H + D_IN] = wfu[0, :D_IN]
    wg0h = np.zeros((128, 2 * D_H), dtype=np.float16)
    wg0h[0:D_H] = wg0h[D_H:2 * D_H] = wfu[0, D_IN:]
    wcx0 = np.zeros((128, D_H), dtype=np.float16)
    wcx0[0:D_IN] = wcx0[D_H:D_H + D_IN] = wc3[0, :D_IN].astype(np.float16)
    # [z1x|z2x] weights, duplicated per batch row-block for base matching
    wcxz1 = np.concatenate([wc3[1, :D_IN], wc3[2, :D_IN]])       # [64, 64]
    wcxz = np.concatenate([wcxz1, wcxz1]).astype(np.float16)     # [128, 64]
    wch = np.concatenate([wc3[0, D_IN:], wc3[1, D_IN:]]).astype(np.float16)
    wch2 = wc3[2, D_IN:].astype(np.float16)               # [64, 64]
    bfu = np.concatenate([bf, bu]).reshape(2 * D_H, 1).astype(np.float32)
    bcb = np.concatenate([bc, bc]).reshape(2 * D_H, 1).astype(np.float32)

    shared = {"at": at8, "wfu": wfu, "wg0x": wg0x, "wg0h": wg0h,
              "wcx0": wcx0, "wcxz": wcxz, "wch": wch, "wch2": wch2,
              "bfu": bfu, "bcb": bcb, "idm": idm}

    z = np.concatenate([x, h], axis=1)                    # [B, 96, 4000]
    zp = np.zeros((B, C, NP), dtype=np.float32)
    zp[:, :, :NN] = z
    in_maps = []
    for core in range(NCORES):
        bs = slice(core * B_LOC, (core + 1) * B_LOC)
        zc = zp[bs]                                       # [2, 96, 4096]
        # node-major fp8 [128, 32, 192]
        zt = _f8(zc.transpose(2, 0, 1).reshape(NCH, 128, BC)
                 .transpose(1, 0, 2))
        # batch-stacked x (rows 0:32, 64:96) and h (rows 0:64, 64:128)
        xal = np.zeros((128, NP), dtype=np.float16)
        hal = np.zeros((128, NP), dtype=np.float16)
        for b in range(B_LOC):
            xal[b * D_H:b * D_H + D_IN] = zc[b, :D_IN]
            hal[b * D_H:(b + 1) * D_H] = zc[b, D_IN:]
        in_maps.append(dict(shared, zt=np.ascontiguousarray(zt),
                            xal=xal, hal=hal))
    return in_maps


def kernel(**inputs):
    global _CACHED_NC, LAST_RESULTS
    inputs = {k: np.asarray(v) for k, v in inputs.items()}
    if _CACHED_NC is None:
        _CACHED_NC = build_program()
    in_maps = _host_prep(**inputs)
    kw = {}
    if TRACE:
        kw = dict(trace=True, tmpdir=TRACE_DIR)
    res = run_bass_kernel_spmd(_CACHED_NC, in_maps,
                               core_ids=list(range(NCORES)), **kw)
    LAST_RESULTS = res
    outs = [res.results[i]["out"] for i in range(NCORES)]
    return np.concatenate(outs, axis=0).astype(np.float32)


if __name__ == "__main__":
    rng = np.random.default_rng(0)
    ins = {
        "x": rng.standard_normal((B, D_IN, NN), dtype=np.float32),
        "h": rng.standard_normal((B, D_H, NN), dtype=np.float32),
        "adj": rng.random((NN, NN), dtype=np.float32) / NN,
        "Wf": rng.standard_normal((D_H, 3 * C), dtype=np.float32) * 0.05,
        "Wu": rng.standard_normal((D_H, 3 * C), dtype=np.float32) * 0.05,
        "Wc": rng.standard_normal((D_H, 3 * C), dtype=np.float32) * 0.05,
        "bf": rng.standard_normal(D_H).astype(np.float32) * 0.05,
        "bu": rng.standard_normal(D_H).astype(np.float32) * 0.05,
        "bc": rng.standard_normal(D_H).astype(np.float32) * 0.05,
    }
    out = kernel(**ins)
    print(out.shape, out.dtype)


# revision 10
# speedup vs baseline: 1.2465x; 1.2465x over previous
"""GCGRU cell (order-2 graph diffusion GRU) Trainium2 Bass kernel.

Strategy: data-parallel over batch (B=16 -> 2 batches per core x 8 cores).
The 4000x4000 adjacency is kept RESIDENT in SBUF as fp8 (adjT scaled by 2^11
into e4m3 range, chunk-major [128, 32 chunks, 4000]), so HBM sees it exactly
once per core (~16MB) instead of once per diffusion stage (4x33MB fp16).

All four graph diffusions run as fp8 DoubleRow matmuls (K=256 per chunk pair)
with the activations as the PE-stationary operand and adjacency slabs moving
512 output columns at a time: z1=(A z), z2=(A z1) for the gates, zc1=(A rh),
zc2=(A zc1) for the candidate. Diffused features carry the 2^11 adjacency
scale in fp16/fp8; conv weights are pre-scaled by 2^-11 host-side, and the
second-order stages descale their PSUM copy by 2^-11 on ScalarE.

Chained diffusions need node-major stationaries, so each stage's PSUM bands
are PE-transposed (128-chunk pieces) and cast to fp8 on DVE. Gate and
candidate 1x1 convs consume channel-major copies (fp16 for the dominant
direct terms, fp8 for the small diffused terms), fused per 512-node band:
sigmoid/tanh on ScalarE, r*h / u*h+(1-u)*c combines on DVE, outputs DMA'd
per band. All input casts/layout transforms are done on host in kernel().
"""

import numpy as np
import ml_dtypes

import concourse.bass as bass
from concourse import bacc
import concourse.mybir as mybir
import concourse.tile as tile
from concourse.bass_utils import run_bass_kernel_spmd

# problem constants
B, D_IN, D_H, NN = 16, 32, 64, 4000
NCORES = 8
B_LOC = B // NCORES          # batches per core
C = D_IN + D_H               # 96 channels into each gate conv
BC = B_LOC * C               # 192 batch-channel columns (b-major)
BH = B_LOC * D_H             # 128 stacked batch-hidden rows
NP = 4096                    # node contraction dim padded to 32 chunks
NCH = NP // 128              # 32 node chunks
NJP = NCH // 2               # 16 DoubleRow chunk pairs
SC = 2048.0                  # adjacency pre-scale (2^11) into fp8 range
BAND = 512                   # node band width for PSUM groups / convs
NBAND = (NN + BAND - 1) // BAND   # 8 bands, last is 416 wide

F8 = mybir.dt.float8e4
F16 = mybir.dt.float16
F32 = mybir.dt.float32
DR = mybir.MatmulPerfMode.DoubleRow
NP8 = ml_dtypes.float8_e4m3


def band_w(bi):
    return min(BAND, NN - bi * BAND)


def build_program():
    nc = bacc.Bacc("TRN2", target_bir_lowering=False, debug=False)

    # ---- DRAM I/O (all host-prepped layouts) ----
    # at[g][p, j, m] = adjT[(4g+j)*128+p, m] * SC, fp8, m < 4000
    at_d = nc.dram_tensor("at", [8, 128, 4, NN], F8, kind="ExternalInput").ap()
    # zt[p, j, c] = concat(x,h)[b, ch, j*128+p] fp8, c = b*96+ch
    zt_d = nc.dram_tensor("zt", [128, NCH, BC], F8, kind="ExternalInput").ap()
    xal_d = nc.dram_tensor("xal", [128, NP], F16, kind="ExternalInput").ap()
    hal_d = nc.dram_tensor("hal", [128, NP], F16, kind="ExternalInput").ap()
    wfu_d = nc.dram_tensor("wfu", [3, C, 2 * D_H], F16, kind="ExternalInput").ap()
    wg0x_d = nc.dram_tensor("wg0x", [128, 2 * D_H], F16, kind="ExternalInput").ap()
    wg0h_d = nc.dram_tensor("wg0h", [128, 2 * D_H], F16, kind="ExternalInput").ap()
    wcx0_d = nc.dram_tensor("wcx0", [128, D_H], F16, kind="ExternalInput").ap()
    wcxz_d = nc.dram_tensor("wcxz", [2 * D_H, D_H], F16,
                            kind="ExternalInput").ap()
    wch_d = nc.dram_tensor("wch", [2 * D_H, D_H], F16, kind="ExternalInput").ap()
    wch2_d = nc.dram_tensor("wch2", [D_H, D_H], F16, kind="ExternalInput").ap()
    bfu_d = nc.dram_tensor("bfu", [2 * D_H, 1], F32, kind="ExternalInput").ap()
    bcb_d = nc.dram_tensor("bcb", [2 * D_H, 1], F32, kind="ExternalInput").ap()
    id_d = nc.dram_tensor("idm", [128, 128], F16, kind="ExternalInput").ap()
    out_d = nc.dram_tensor("out", [B_LOC, D_H, NN], F32,
                           kind="ExternalOutput").ap()

    with tile.TileContext(nc) as tc:
        _body(tc, at_d, zt_d, xal_d, hal_d, wfu_d, wg0x_d, wg0h_d,
              wcx0_d, wcxz_d, wch_d, wch2_d, bfu_d, bcb_d, id_d, out_d)
    nc.compile()
    return nc


def _body(tc, at_d, zt_d, xal_d, hal_d, wfu_d, wg0x_d, wg0h_d,
          wcx0_d, wcxz_d, wch_d, wch2_d, bfu_d, bcb_d, id_d, out_d):
    nc = tc.nc
    SIG = mybir.ActivationFunctionType.Sigmoid
    TANH = mybir.ActivationFunctionType.Tanh
    COPY = mybir.ActivationFunctionType.Copy

    with (
        tc.tile_pool(name="const", bufs=1) as cpool,     # weights/bias/idm/at
        tc.tile_pool(name="ztp", bufs=2) as ztpool,      # zt8 <-> z1T8 reuse
        tc.tile_pool(name="act8", bufs=1) as a8pool,     # persistent fp8 acts
        tc.tile_pool(name="perst", bufs=1) as ppool,     # persistent fp16 acts
        tc.tile_pool(name="bnd", bufs=6) as bpool,       # band staging tiles
        tc.tile_pool(name="stg", bufs=3) as stpool,      # combine staging
        tc.tile_pool(name="psmm", bufs=4, space="PSUM") as mmpool,
        tc.tile_pool(name="pscv", bufs=2, space="PSUM") as cvpool,
        tc.tile_pool(name="psx", bufs=2, space="PSUM") as txpool,
    ):
        # ---- persistent loads ----
        idm = cpool.tile([128, 128], F16, tag="idm")
        nc.sync.dma_start(out=idm[:], in_=id_d[:])
        wfu = [cpool.tile([C, 2 * D_H], F16, tag=f"wfu{k}", name=f"wfu{k}")
               for k in range(3)]
        for k in range(3):
            nc.scalar.dma_start(out=wfu[k][:], in_=wfu_d[k])
        wg0x = cpool.tile([128, 2 * D_H], F16, tag="wg0x")
        nc.sync.dma_start(out=wg0x[:], in_=wg0x_d[:])
        wg0h = cpool.tile([128, 2 * D_H], F16, tag="wg0h")
        nc.sync.dma_start(out=wg0h[:], in_=wg0h_d[:])
        wcx0 = cpool.tile([128, D_H], F16, tag="wcx0")
        nc.sync.dma_start(out=wcx0[:], in_=wcx0_d[:])
        wcxz = cpool.tile([2 * D_H, D_H], F16, tag="wcxz")
        nc.sync.dma_start(out=wcxz[:], in_=wcxz_d[:])
        wch = cpool.tile([2 * D_H, D_H], F16, tag="wch")
        nc.sync.dma_start(out=wch[:], in_=wch_d[:])
        wch2 = cpool.tile([D_H, D_H], F16, tag="wch2")
        nc.sync.dma_start(out=wch2[:], in_=wch2_d[:])
        bfu = cpool.tile([2 * D_H, 1], F32, tag="bfu")
        nc.sync.dma_start(out=bfu[:], in_=bfu_d[:])
        bcb = cpool.tile([2 * D_H, 1], F32, tag="bcb")
        nc.sync.dma_start(out=bcb[:], in_=bcb_d[:])
        xall = ppool.tile([128, NP], F16, tag="xall")
        nc.scalar.dma_start(out=xall[:], in_=xal_d[:])
        hall = ppool.tile([128, NP], F16, tag="hall")
        nc.scalar.dma_start(out=hall[:], in_=hal_d[:])

        zt8 = ztpool.tile([128, NCH * BC], F8, tag="ztn", name="zt8")
        nc.sync.dma_start(
            out=zt8[:, :].rearrange("p (j c) -> p j c", j=NCH), in_=zt_d[:])

        # resident adjacency, 8 slab DMAs alternating HWDGE rings
        at8 = cpool.tile([128, NCH * NN], F8, tag="at8")
        at3 = at8[:, :].rearrange("p (j m) -> p j m", j=NCH)
        for g in range(8):
            eng = nc.sync if g % 2 == 0 else nc.scalar
            eng.dma_start(out=at3[:, 4 * g:4 * g + 4, :], in_=at_d[g])

        # persistent activations
        u_st = ppool.tile([BH, NP], F16, tag="u_st")     # u, rows b*64
        z1cm8 = [a8pool.tile([C, NP], F8, tag=f"z1cm{b}", name=f"z1cm{b}")
                 for b in range(B_LOC)]
        # candidate diffused-x features, rows [b0z1x|b0z2x|b1z1x|b1z2x]
        candx = a8pool.tile([128, NP], F8, tag="candx")
        rhz = [a8pool.tile([2 * D_H, NP], F8, tag=f"rhz{b}", name=f"rhz{b}")
               for b in range(B_LOC)]                    # [rh|zc1] rows
        rhT8 = a8pool.tile([128, NCH * BH], F8, tag="rhT8")
        zc1T8 = a8pool.tile([128, NCH * BH], F8, tag="zc1T8")
        z1T8 = ztpool.tile([128, NCH * BC], F8, tag="ztn", name="z1T8")

        zt3 = zt8[:, :].rearrange("p (j c) -> p j c", j=NCH)
        z1T3 = z1T8[:, :].rearrange("p (j c) -> p j c", j=NCH)
        rhT3 = rhT8[:, :].rearrange("p (j c) -> p j c", j=NCH)
        zc1T3 = zc1T8[:, :].rearrange("p (j c) -> p j c", j=NCH)

        # zero the padded node rows (4000:4096 live in chunk 31) of the
        # on-chip-built stationaries so later contractions see zeros; the
        # band transposes overwrite rows 0:32 with real data afterwards
        nc.vector.memset(z1T3[:, 31, :], 0.0)
        nc.vector.memset(rhT3[:, 31, :], 0.0)
        nc.vector.memset(zc1T3[:, 31, :], 0.0)

        def diffuse_bc(src3, bands, psname):
            """DoubleRow diffusion, [*,*,BC]-layout stationary; returns
            per-(band, batch) psum tiles [C, w] accumulated over all nodes."""
            pss = {}
            for bi in bands:
                for b in range(B_LOC):
                    pss[bi, b] = mmpool.tile([C, BAND], F32, tag="mm",
                                             name=f"{psname}_{bi}_{b}")
            for jp in range(NJP):
                for b in range(B_LOC):
                    lhsT = src3[:, 2 * jp:2 * jp + 2, b * C:(b + 1) * C]
                    for bi in bands:
                        w = band_w(bi)
                        nc.tensor.matmul(
                            pss[bi, b][:, 0:w], lhsT=lhsT,
                            rhs=at3[:, 2 * jp:2 * jp + 2,
                                    bi * BAND:bi * BAND + w],
                            start=(jp == 0), stop=(jp == NJP - 1),
                            perf_mode=DR)
            return pss

        def diffuse_bh(src3, bands, psname):
            """DoubleRow diffusion, [*,*,BH]-layout stationary (both batches
            in one tile); returns per-band psum tiles [BH, w]."""
            pss = {}
            for bi in bands:
                pss[bi] = mmpool.tile([BH, BAND], F32, tag="mm",
                                      name=f"{psname}_{bi}")
            for jp in range(NJP):
                lhsT = src3[:, 2 * jp:2 * jp + 2, :]
                for bi in bands:
                    w = band_w(bi)
                    nc.tensor.matmul(
                        pss[bi][:, 0:w], lhsT=lhsT,
                        rhs=at3[:, 2 * jp:2 * jp + 2, bi * BAND:bi * BAND + w],
                        start=(jp == 0), stop=(jp == NJP - 1), perf_mode=DR)
            return pss

        def transpose_band(srcT, r0, rows, bi, dst3, dcol0, dcols):
            """PE-transpose band staging srcT[r0:r0+rows, 0:w] (f16) into fp8
            node-major dst3[:, j, dcol0:dcol0+dcols] chunk pieces."""
            w = band_w(bi)
            m0 = bi * BAND
            rsl = slice(r0, r0 + rows)
            for j in range(m0 // 128, (m0 + w + 127) // 128):
                off = j * 128 - m0
                wj = min(128, w - off)
                pt = txpool.tile([128, rows], F16, tag="tx", name=f"tp{j}")
                nc.tensor.transpose(pt[0:wj, :], srcT[rsl, off:off + wj],
                                    idm[rsl, rsl])
                nc.vector.tensor_copy(
                    out=dst3[0:wj, j, dcol0:dcol0 + dcols], in_=pt[0:wj, :])

        # ================= stage 1: z1 = (A*SC) z =================
        for q in range(NBAND // 2):
            bands = (2 * q, 2 * q + 1)
            pss = diffuse_bc(zt3, bands, "ps1")
            for bi in bands:
                w = band_w(bi)
                msl = slice(bi * BAND, bi * BAND + w)
                for b in range(B_LOC):
                    ps = pss[bi, b]
                    z1b = bpool.tile([C, BAND], F16, tag="bnd", name="z1b")
                    nc.scalar.activation(z1b[:, 0:w], ps[:, 0:w], COPY)
                    nc.vector.tensor_copy(out=z1cm8[b][:, msl], in_=ps[:, 0:w])
                    nc.vector.tensor_copy(
                        out=candx[b * D_H:b * D_H + D_IN, msl],
                        in_=ps[0:D_IN, 0:w])
                    transpose_band(z1b, 0, C, bi, z1T3, b * C, C)

        # ============ stage 2: z2 = (A*SC) z1s, gates, rh ============
        for q in range(NBAND // 2):
            bands = (2 * q, 2 * q + 1)
            pss = diffuse_bc(z1T3, bands, "ps2")
            for bi in bands:
                w = band_w(bi)
                msl = slice(bi * BAND, bi * BAND + w)
                rst = bpool.tile([BH, BAND], F16, tag="bnd", name="rst")
                for b in range(B_LOC):
                    ps = pss[bi, b]
                    # z2s = 2^11 z2 (psum holds 2^22 z2)
                    z2b = bpool.tile([C, BAND], F16, tag="bnd", name="z2b")
                    nc.scalar.activation(z2b[:, 0:w], ps[:, 0:w], COPY,
                                         scale=1.0 / SC)
                    nc.vector.tensor_scalar_mul(
                        candx[b * D_H + D_IN:b * D_H + 2 * D_IN, msl],
                        ps[0:D_IN, 0:w], 1.0 / SC)
                    # gate conv: r|u preact = Wfu0 z + Wfu1' z1s + Wfu2' z2s
                    psg = cvpool.tile([2 * D_H, BAND], F32, tag="cv",
                                      name="psg")
                    gt = ((wg0x[b * D_H:b * D_H + D_IN, :],
                           xall[b * D_H:b * D_H + D_IN, msl]),
                          (wg0h[b * D_H:(b + 1) * D_H, :],
                           hall[b * D_H:(b + 1) * D_H, msl]),
                          (wfu[1], z1cm8[b][:, msl]),
                          (wfu[2], z2b[:, 0:w]))
                    for k, (wt, rhs) in enumerate(gt):
                        nc.tensor.matmul(psg[:, 0:w], lhsT=wt, rhs=rhs,
                                         start=(k == 0),
                                         stop=(k == len(gt) - 1))
                    rows = slice(b * D_H, (b + 1) * D_H)
                    nc.scalar.activation(rst[rows, 0:w], psg[0:D_H, 0:w],
                                         SIG, bias=bfu[0:D_H, :])
                    nc.scalar.activation(u_st[rows, msl],
                                         psg[D_H:2 * D_H, 0:w], SIG,
                                         bias=bfu[D_H:2 * D_H, :])
                if True:  # both batches' r now staged batch-stacked in rst
                    rhb = bpool.tile([BH, BAND], F16, tag="bnd", name="rhb")
                    nc.vector.tensor_mul(out=rhb[:, 0:w], in0=rst[:, 0:w],
                                         in1=hall[:, msl])
                    for b in range(B_LOC):
                        nc.vector.tensor_copy(
                            out=rhz[b][0:D_H, msl],
                            in_=rhb[b * D_H:(b + 1) * D_H, 0:w])
                    transpose_band(rhb, 0, BH, bi, rhT3, 0, BH)

        # ================= stage 3: zc1 = (A*SC) rh =================
        for hh in range(NBAND // 4):
            bands = tuple(range(4 * hh, 4 * hh + 4))
            pss = diffuse_bh(rhT3, bands, "ps3")
            for bi in bands:
                w = band_w(bi)
                msl = slice(bi * BAND, bi * BAND + w)
                zc1b = bpool.tile([BH, BAND], F16, tag="bnd", name="zc1b")
                nc.scalar.activation(zc1b[:, 0:w], pss[bi][:, 0:w], COPY)
                for b in range(B_LOC):
                    nc.vector.tensor_copy(
                        out=rhz[b][D_H:2 * D_H, msl],
                        in_=pss[bi][b * D_H:(b + 1) * D_H, 0:w])
                transpose_band(zc1b, 0, BH, bi, zc1T3, 0, BH)

        # ================ stage 4: zc2, candidate, output ================
        for bands in ((0, 1, 2, 3), (4, 5), (6,), (7,)):
            pss = diffuse_bh(zc1T3, bands, "ps4")
            for bi in bands:
                w = band_w(bi)
                msl = slice(bi * BAND, bi * BAND + w)
                psc = cvpool.tile([BH, BAND], F32, tag="cv", name="psc")
                for b in range(B_LOC):
                    rows = slice(b * D_H, (b + 1) * D_H)
                    zc2b = bpool.tile([D_H, BAND], F16, tag="bnd",
                                      name="zc2b")
                    nc.scalar.activation(zc2b[:, 0:w], pss[bi][rows, 0:w],
                                         COPY, scale=1.0 / SC)
                    terms = ((wcx0[b * D_H:b * D_H + D_IN, :],
                              xall[b * D_H:b * D_H + D_IN, msl]),
                             (wcxz[b * D_H:(b + 1) * D_H, :],
                              candx[b * D_H:(b + 1) * D_H, msl]),
                             (wch, rhz[b][:, msl]),
                             (wch2, zc2b[:, 0:w]))
                    for k, (wt, rhs) in enumerate(terms):
                        nc.tensor.matmul(psc[rows, 0:w], lhsT=wt, rhs=rhs,
                                         start=(k == 0),
                                         stop=(k == len(terms) - 1))
                cst = bpool.tile([BH, BAND], F16, tag="bnd", name="cst")
                nc.scalar.activation(cst[:, 0:w], psc[:, 0:w], TANH,
                                     bias=bcb[:, :])
                t1 = stpool.tile([BH, BAND], F32, tag="cmb", name="t1")
                nc.vector.tensor_sub(out=t1[:, 0:w], in0=hall[:, msl],
                                     in1=cst[:, 0:w])
                nc.vector.tensor_mul(out=t1[:, 0:w], in0=u_st[:, msl],
                                     in1=t1[:, 0:w])
                ost = stpool.tile([BH, BAND], F32, tag="cmb", name="ost")
                nc.vector.tensor_add(out=ost[:, 0:w], in0=cst[:, 0:w],
                                     in1=t1[:, 0:w])
                for b in range(B_LOC):
                    nc.scalar.dma_start(
                        out=out_d[b][:, msl],
                        in_=ost[b * D_H:(b + 1) * D_H, 0:w])


# ---- host-side driver ----
_CACHED_NC = None
TRACE = False
TRACE_DIR = None
LAST_RESULTS = None


def _f8(a):
    return np.clip(a, -240.0, 240.0).astype(NP8)


def _host_prep(x, h, adj, Wf, bf, Wu, bu, Wc, bc):
    # adjacency: transpose, scale, pad contraction rows to 4096, fp8,
    # chunk-major groups of 4
    at = np.zeros((NP, NN), dtype=np.float32)
    at[:NN, :] = adj.T * SC
    at8 = _f8(at).reshape(8, 4, 128, NN).transpose(0, 2, 1, 3)
    at8 = np.ascontiguousarray(at8)                       # [8, 128, 4, 4000]
    idm = np.eye(128, dtype=np.float16)

    def wsplit(W, s):  # [D_H, 3C] -> [3, C, D_H] with per-order scales
        WT = np.ascontiguousarray(W.T.reshape(3, C, D_H)).astype(np.float32)
        return WT * np.asarray(s, np.float32)[:, None, None]

    wf3 = wsplit(Wf, [1, 1 / SC, 1 / SC])
    wu3 = wsplit(Wu, [1, 1 / SC, 1 / SC])
    wc3 = wsplit(Wc, [1, 1 / SC, 1 / SC])
    # gate weights packed [96, f64|u64] per order
    wfu = np.concatenate([wf3, wu3], axis=2).astype(np.float16)  # [3, 96, 128]
    # direct-term gate weights split x/h, duplicated per batch row-block so
    # matmul stationary/moving base partitions match (b0 rows 0:*, b1 at 64:*)
    wg0x = np.zeros((128, 2 * D_H), dtype=np.float16)
    wg0x[0:D_IN] = wg0x[D_H:D_H + D_IN] = wfu[0, :D_IN]
    wg0h = np.zeros((128, 2 * D_H), dtype=np.float16)
    wg0h[0:D_H] = wg0h[D_H:2 * D_H] = wfu[0, D_IN:]
    wcx0 = np.zeros((128, D_H), dtype=np.float16)
    wcx0[0:D_IN] = wcx0[D_H:D_H + D_IN] = wc3[0, :D_IN].astype(np.float16)
    # [z1x|z2x] weights, duplicated per batch row-block for base matching
    wcxz1 = np.concatenate([wc3[1, :D_IN], wc3[2, :D_IN]])       # [64, 64]
    wcxz = np.concatenate([wcxz1, wcxz1]).astype(np.float16)     # [128, 64]
    wch = np.concatenate([wc3[0, D_IN:], wc3[1, D_IN:]]).astype(np.float16)
    wch2 = wc3[2, D_IN:].astype(np.float16)               # [64, 64]
    bfu = np.concatenate([bf, bu]).reshape(2 * D_H, 1).astype(np.float32)
    bcb = np.concatenate([bc, bc]).reshape(2 * D_H, 1).astype(np.float32)

    shared = {"at": at8, "wfu": wfu, "wg0x": wg0x, "wg0h": wg0h,
              "wcx0": wcx0, "wcxz": wcxz, "wch": wch, "wch2": wch2,
              "bfu": bfu, "bcb": bcb, "idm": idm}

    z = np.concatenate([x, h], axis=1)                    # [B, 96, 4000]
    zp = np.zeros((B, C, NP), dtype=np.float32)
    zp[:, :, :NN] = z
    in_maps = []
    for core in range(NCORES):
        bs = slice(core * B_LOC, (core + 1) * B_LOC)
        zc = zp[bs]                                       # [2, 96, 4096]
        # node-major fp8 [128, 32, 192]
        zt = _f8(zc.transpose(2, 0, 1).reshape(NCH, 128, BC)
                 .transpose(1, 0, 2))
        # batch-stacked x (rows 0:32, 64:96) and h (rows 0:64, 64:128)
        xal = np.zeros((128, NP), dtype=np.float16)
        hal = np.zeros((128, NP), dtype=np.float16)
        for b in range(B_LOC):
            xal[b * D_H:b * D_H + D_IN] = zc[b, :D_IN]
            hal[b * D_H:(b + 1) * D_H] = zc[b, D_IN:]
        in_maps.append(dict(shared, zt=np.ascontiguousarray(zt),
                            xal=xal, hal=hal))
    return in_maps


def kernel(**inputs):
    global _CACHED_NC, LAST_RESULTS
    inputs = {k: np.asarray(v) for k, v in inputs.items()}
    if _CACHED_NC is None:
        _CACHED_NC = build_program()
    in_maps = _host_prep(**inputs)
    kw = {}
    if TRACE:
        kw = dict(trace=True, tmpdir=TRACE_DIR)
    res = run_bass_kernel_spmd(_CACHED_NC, in_maps,
                               core_ids=list(range(NCORES)), **kw)
    LAST_RESULTS = res
    outs = [res.results[i]["out"] for i in range(NCORES)]
    return np.concatenate(outs, axis=0).astype(np.float32)


if __name__ == "__main__":
    rng = np.random.default_rng(0)
    ins = {
        "x": rng.standard_normal((B, D_IN, NN), dtype=np.float32),
        "h": rng.standard_normal((B, D_H, NN), dtype=np.float32),
        "adj": rng.random((NN, NN), dtype=np.float32) / NN,
        "Wf": rng.standard_normal((D_H, 3 * C), dtype=np.float32) * 0.05,
        "Wu": rng.standard_normal((D_H, 3 * C), dtype=np.float32) * 0.05,
        "Wc": rng.standard_normal((D_H, 3 * C), dtype=np.float32) * 0.05,
        "bf": rng.standard_normal(D_H).astype(np.float32) * 0.05,
        "bu": rng.standard_normal(D_H).astype(np.float32) * 0.05,
        "bc": rng.standard_normal(D_H).astype(np.float32) * 0.05,
    }
    out = kernel(**ins)
    print(out.shape, out.dtype)
